# revision 67
# baseline (speedup 1.0000x reference)
"""Trainium2 Bass kernel for 2-layer GAT node classification (50K nodes, 800K edges).

Design (vs. the gather-everything baseline):
  - Layer 1 runs with NO collective: x is a full input on every core, so each
    core computes the FULL node table locally (replicated dense phase).
  - Node features travel in a per-head rotated basis ("y-space"): an
    orthogonal Householder transform with first row = a_src/||a_src|| is
    folded into the dense weights on the host, so y[h*32] IS the attention
    source logit. Table rows shrink to 256B (the dma_gather minimum), halving
    gather traffic; messages aggregate linearly in y-space and a per-round
    128x128 rotate-back matmul restores h-space before the nonlinear
    leaky-relu.
  - The layer-2 table is distributed by a 4-chunk AllGather over round
    blocks (small first chunk), overlapping the collective with the layer-1
    edge phase; the chunk-permuted row layout is baked into the host-built
    gather indices.
  - Per-core node order is rotated (own block first) so the SPMD program
    extracts own-destination data at fixed positions.
  - Edge phase is per-destination-round: dma_gathers per (round, window),
    per-round PSUM accumulation via paired identity matmuls, leaky/exp on
    the Activation engine (Prelu+Exp+Ln+Copy share one act table), alpha
    duplicated into pairs so the message multiply hits the DVE 2x mode,
    normalization fused per round, log-softmax Ln batched across rounds.
"""
import sys

sys.path.insert(0, "/opt/trn_rl_repo")

import numpy as np

import concourse.bacc as bacc
import concourse.tile as tile
import concourse.mybir as mybir
from concourse.bass_utils import run_bass_kernel_spmd

P = 128
NCORES = 8
F_IN = 128
H = 4
C = 32
HC = 128
NCLS = 40
NEG = 0.2
WIN = 32768
RING = 16384  # SWDGE ring; >1024 idxs per call hangs the device
MAXCH = (RING // 16) // P  # max chunks per dma_gather call

f32 = mybir.dt.float32
f16 = mybir.dt.float16
u16 = mybir.dt.uint16
i16 = mybir.dt.int16
u8 = mybir.dt.uint8
f8 = mybir.dt.float8e4  # e4m3

LAST_EXEC_NS = None
import os as _os
NO_PRELU = _os.environ.get("V2_NO_PRELU", "0") == "1"
NO_PAIR = _os.environ.get("V2_NO_PAIR", "0") == "1"
NO_LN = _os.environ.get("V2_NO_LN", "0") == "1"
NO_GATHER = _os.environ.get("V2_NO_GATHER", "0") == "1"
NO_COLL = _os.environ.get("V2_NO_COLL", "0") == "1"



# ---------------------------------------------------------------------------
# host preprocessing
# ---------------------------------------------------------------------------

def _cumcount(keys):
    n = len(keys)
    if n == 0:
        return np.zeros(0, dtype=np.int64)
    first = np.ones(n, dtype=bool)
    first[1:] = keys[1:] != keys[:-1]
    idx = np.arange(n)
    start = np.maximum.accumulate(np.where(first, idx, 0))
    return idx - start


def _build_grids(src_row, dst_newid, npc, tpc, table_rows,
                 lo_cut=None, lo_frac=0.5):
    """Per-core slot grids for one layer.

    src_row: [Etot, NCORES] table row of the source as seen by each core
             (layer 1: rotated; layer 2: same global row for all cores).
    dst_newid: [Etot] global new id of the destination.
    lo_cut: rows >= lo_cut are hi-only even if < WIN (layer 2: rows of the
            last AG chunk are not yet valid when the lo/A stream gathers).
    lo_frac: target fraction of a node's edges assigned to the lo stream.
    Returns KL, KH [tpc] (common across cores) and per-core packed slot
    arrays (values = window-relative table rows).
    """
    hi_base = max(0, table_rows - WIN)
    if lo_cut is None:
        lo_cut = WIN
    dst_core = dst_newid // npc
    r_e = (dst_newid % npc) // P
    lane_e = dst_newid % P

    kl_counts = np.zeros((NCORES, tpc, P), dtype=np.int64)
    kh_counts = np.zeros((NCORES, tpc, P), dtype=np.int64)
    per_core = []
    for c in range(NCORES):
        m = dst_core == c
        rows = src_row[m, c] if src_row.ndim == 2 else src_row[m]
        d_r = r_e[m]
        d_lane = lane_e[m]
        cat = np.full(len(rows), 2, dtype=np.int8)  # flex
        cat[rows < hi_base] = 0  # lo only
        cat[rows >= lo_cut] = 1  # hi only
        dkey = d_r * P + d_lane
        o = np.argsort(dkey, kind="stable")
        rows, d_r, d_lane, cat, dkey = rows[o], d_r[o], d_lane[o], cat[o], dkey[o]
        ndeg = np.bincount(dkey, minlength=tpc * P)
        nlo = np.bincount(dkey[cat == 0], minlength=tpc * P)
        nhi = np.bincount(dkey[cat == 1], minlength=tpc * P)
        tgt = (ndeg * lo_frac + 0.5).astype(np.int64)
        kl_node = np.maximum(nlo, np.minimum(ndeg - nhi, tgt))
        flex_rank = np.zeros(len(rows), dtype=np.int64)
        mflex = cat == 2
        flex_rank[mflex] = _cumcount(dkey[mflex])
        to_lo = (cat == 0) | (mflex & (flex_rank < (kl_node - nlo)[dkey]))
        k_slot = np.zeros(len(rows), dtype=np.int64)
        for mm in (to_lo, ~to_lo):
            k_slot[mm] = _cumcount(dkey[mm])
        kl_counts[c] = kl_node.reshape(tpc, P)
        kh_counts[c] = (ndeg - kl_node).reshape(tpc, P)
        per_core.append((rows, d_r, d_lane, to_lo, k_slot))

    KL = kl_counts.max(axis=(0, 2)).astype(np.int64)
    KH = kh_counts.max(axis=(0, 2)).astype(np.int64)
    cumKL = np.concatenate([[0], np.cumsum(KL)])
    cumKH = np.concatenate([[0], np.cumsum(KH)])
    CL, CH = int(cumKL[-1]), int(cumKH[-1])

    DUM_LO = 0
    DUM_HI = table_rows - 1 - hi_base
    slots_lo = np.full((NCORES, CL * P), DUM_LO, dtype=np.int64)
    slots_hi = np.full((NCORES, CH * P), DUM_HI, dtype=np.int64)
    for c in range(NCORES):
        rows, d_r, d_lane, to_lo, k_slot = per_core[c]
        pos_lo = (cumKL[d_r] + k_slot) * P + d_lane
        pos_hi = (cumKH[d_r] + k_slot) * P + d_lane
        slots_lo[c, pos_lo[to_lo]] = rows[to_lo]
        slots_hi[c, pos_hi[~to_lo]] = rows[~to_lo] - hi_base
    return KL, KH, slots_lo, slots_hi


def _build_grids_fixed(src_row, dst_newid, npc, tpc, cut, dum_b):
    """Two-category grid with FIXED assignment: A = row < cut, B = else.
    Slot values: A-stream = row; B-stream = row - cut."""
    dst_core = dst_newid // npc
    r_e = (dst_newid % npc) // P
    lane_e = dst_newid % P
    ka_c = np.zeros((NCORES, tpc, P), dtype=np.int64)
    kb_c = np.zeros((NCORES, tpc, P), dtype=np.int64)
    per_core = []
    for c in range(NCORES):
        m = dst_core == c
        rows = src_row[m]
        d_r = r_e[m]
        d_lane = lane_e[m]
        dkey = d_r * P + d_lane
        o = np.argsort(dkey, kind="stable")
        rows, dkey = rows[o], dkey[o]
        isA = rows < cut
        ka_c[c] = np.bincount(dkey[isA], minlength=tpc * P).reshape(tpc, P)
        kb_c[c] = np.bincount(dkey[~isA], minlength=tpc * P).reshape(tpc, P)
        k_slot = np.zeros(len(rows), dtype=np.int64)
        for mm in (isA, ~isA):
            k_slot[mm] = _cumcount(dkey[mm])
        per_core.append((rows, dkey, isA, k_slot))
    KA = ka_c.max(axis=(0, 2)).astype(np.int64)
    KB = kb_c.max(axis=(0, 2)).astype(np.int64)
    cumKA = np.concatenate([[0], np.cumsum(KA)])
    cumKB = np.concatenate([[0], np.cumsum(KB)])
    slots_a = np.full((NCORES, int(cumKA[-1]) * P), 0, dtype=np.int64)
    slots_b = np.full((NCORES, int(cumKB[-1]) * P), dum_b, dtype=np.int64)
    for c in range(NCORES):
        rows, dkey, isA, k_slot = per_core[c]
        d_r = dkey // P
        lane = dkey % P
        pa = (cumKA[d_r] + k_slot) * P + lane
        pb = (cumKB[d_r] + k_slot) * P + lane
        slots_a[c, pa[isA]] = rows[isA]
        slots_b[c, pb[~isA]] = rows[~isA] - cut
    return KA, KB, slots_a, slots_b


def _pack(slots):
    """[NCORES, n_slots] -> [NCORES, 128, n_slots//16] int16 idx layout."""
    ncols = slots.shape[1] // 16
    if ncols == 0:
        return np.zeros((NCORES, 128, 0), np.int16)
    a = slots.reshape(NCORES, ncols, 16).transpose(0, 2, 1)
    a = a.astype(np.uint16).view(np.int16)
    return np.tile(a, (1, 8, 1))


def _preprocess(x, edge_index, n_real):
    n_tiles = -(-(n_real + 1) // P)
    n_tiles = -(-n_tiles // NCORES) * NCORES
    npad = n_tiles * P
    tpc = n_tiles // NCORES
    npc = tpc * P
    # partition-major table: row = 1 + lane*n_tiles + gtile (dummy rows at
    # both ends).  A multi-tile store is then contiguous per partition.
    table_rows = 1 + npad + 1
    assert table_rows <= 2 * WIN, "two int16 windows must cover the table"

    src0 = np.asarray(edge_index[0]).astype(np.int64)
    dst0 = np.asarray(edge_index[1]).astype(np.int64)

    deg = np.bincount(dst0, minlength=npad).astype(np.int64)
    deg[:n_real] += 1
    order = np.argsort(deg, kind="stable")
    pos = np.empty(npad, dtype=np.int64)
    pos[order] = np.arange(npad)
    tile_of = pos // P
    lane_of = pos % P
    r_of = tile_of // NCORES
    c_of = tile_of % NCORES
    new_id = c_of * npc + r_of * P + lane_of  # old -> global new id

    all_src = np.concatenate([new_id[src0], new_id[:n_real]])
    all_dst = np.concatenate([new_id[dst0], new_id[:n_real]])

    # layer-1 source rows: rotated per core (own block first), p-major
    blk = all_src // npc
    s_r = (all_src % npc) // P
    s_l = all_src % P
    rot_rows = np.empty((len(all_src), NCORES), dtype=np.int64)
    for c in range(NCORES):
        rot_rows[:, c] = 1 + s_l * n_tiles + ((blk - c) % NCORES) * tpc + s_r
    KL1, KH1, sl1, sh1 = _build_grids(rot_rows, all_dst, npc, tpc, table_rows)
    # layer-2 source rows: chunked-AllGather layout. The AG runs in NCH
    # chunks over round-blocks; chunk k of every core lands rank-major at
    # base_k. row(c, r, l) = 128 + base_k + c*rows_k + (r - r0_k)*128 + l.
    # chunks 0..NCH-2 must fit below WIN so the A stream (which gathers from
    # window [0, WIN) before the last AG lands) can reach them
    a_tiles = min(tpc, (WIN - 1) // (P * NCORES))
    if tpc > a_tiles:
        b0 = max(1, a_tiles // 4)
        b1 = max(b0 + 1, (a_tiles * 4) // 7)
        bounds = np.unique(np.array(
            [0, b0, b1, a_tiles, (a_tiles + tpc + 1) // 2, tpc]))
    else:
        bounds = np.unique(np.array(
            [0, max(1, tpc // 4), max(2, (4 * tpc) // 7), tpc]))
    NCH = len(bounds) - 1
    src_c = all_src // npc
    src_r = (all_src % npc) // P
    src_l = all_src % P
    chunk_of = np.searchsorted(bounds, src_r, side="right") - 1
    tiles_k = bounds[1:] - bounds[:-1]
    rows_k = tiles_k * P
    base_k = np.concatenate([[0], np.cumsum(rows_k * NCORES)])
    # p-major within each (chunk, core) block: local row = lane*tiles + rr
    l2_rows = (1 + base_k[chunk_of] + src_c * rows_k[chunk_of]
               + src_l * tiles_k[chunk_of] + (src_r - bounds[chunk_of]))
    cut = 1 + a_tiles * P * NCORES  # rows gatherable by the A stream
    assert cut <= WIN
    KL2, KH2, sl2, sh2 = _build_grids(
        l2_rows, all_dst, npc, tpc, table_rows, lo_cut=cut, lo_frac=0.62)

    idx1 = np.concatenate([_pack(sl1), _pack(sh1)], axis=2)
    idx2 = np.concatenate([_pack(sl2), _pack(sh2)], axis=2)

    return dict(
        npad=npad, npc=npc, tpc=tpc, table_rows=table_rows,
        ag_bounds=bounds, cut=cut,
        KL1=KL1, KH1=KH1, KL2=KL2, KH2=KH2,
        idx1=np.ascontiguousarray(idx1), idx2=np.ascontiguousarray(idx2),
        new_id=new_id, n_real=n_real,
    )


def _yfold(a_src):
    """Per-head transform T = D @ Q_house (y = T h, y0 = a_src . h) and its
    inverse R = T^{-1} as 128x128 block-diagonal f32 matrices."""
    a = np.asarray(a_src, np.float32)
    T = np.zeros((HC, HC), dtype=np.float64)
    R = np.zeros((HC, HC), dtype=np.float64)
    for h in range(H):
        ah = a[h].astype(np.float64)
        na = np.linalg.norm(ah)
        ahat = ah / na
        v = ahat - np.eye(C)[0]
        if np.linalg.norm(v) < 1e-12:
            Q = np.eye(C)
        else:
            v = v / np.linalg.norm(v)
            Q = np.eye(C) - 2.0 * np.outer(v, v)
        # Q is symmetric orthogonal with Q[0,:] = ahat
        Qo = Q.copy()
        Qo[0, :] = ahat  # guard sign: householder gives exactly this row
        D = np.eye(C)
        D[0, 0] = na
        Th = D @ Qo
        Rh = Qo.T @ np.diag([1.0 / na] + [1.0] * (C - 1))
        T[h * C:(h + 1) * C, h * C:(h + 1) * C] = Th
        R[h * C:(h + 1) * C, h * C:(h + 1) * C] = Rh
    return T.astype(np.float32), R.astype(np.float32)


def _wfull(W, a_src, a_dst):
    """[Wf @ T.T | Wf @ Wad] (132 cols, f16) plus rotate-back R (f16)."""
    W = np.asarray(W, dtype=np.float32)
    fin = W.shape[0]
    Wf = W.reshape(fin, HC)
    T, R = _yfold(a_src)
    Wad = np.zeros((HC, H), dtype=np.float32)
    for h in range(H):
        Wad[h * C:(h + 1) * C, h] = np.asarray(a_dst, np.float32)[h]
    out = np.concatenate([Wf @ T.T, Wf @ Wad], axis=1)  # [fin, 132]
    return (np.ascontiguousarray(out.astype(np.float16)),
            np.ascontiguousarray(R.astype(np.float16)))


def _dummy_row():
    row = np.zeros(128, dtype=np.float16)
    for h in range(H):
        row[h * C] = -60000.0
    return row.view(np.uint16)[None, :]


def _dummy_row8():
    import ml_dtypes
    row = np.zeros(128, dtype=ml_dtypes.float8_e4m3fn)
    for h in range(H):
        row[h * C] = -448.0
    return row.view(np.uint8)[None, :]


# ---------------------------------------------------------------------------
# device program
# ---------------------------------------------------------------------------

def _build(st, b1_zero, b2_zero):
    npc, tpc = st["npc"], st["tpc"]
    npad = st["npad"]
    table_rows = st["table_rows"]
    hi_base = max(0, table_rows - WIN)
    n_tiles = npad // P
    KLs = {1: st["KL1"], 2: st["KL2"]}
    KHs = {1: st["KH1"], 2: st["KH2"]}
    ncols = {1: st["idx1"].shape[2], 2: st["idx2"].shape[2]}
    locol = {
        1: 8 * int(st["KL1"].sum()),
        2: 8 * int(st["KL2"].sum()),
    }
    Kmax = max(
        int((st["KL1"] + st["KH1"]).max()), int((st["KL2"] + st["KH2"]).max())
    )
    idxcols = max(ncols[1], ncols[2], 16)

    nc = bacc.Bacc(None, target_bir_lowering=False,
                   dynamic_dma_scratch_size=RING)

    xT_in = nc.dram_tensor("xT", [F_IN, npad], f16, kind="ExternalInput")
    rot1_in = nc.dram_tensor("rot1", [HC, HC], f16, kind="ExternalInput")
    rot2_in = nc.dram_tensor("rot2", [HC, HC], f16, kind="ExternalInput")
    idx1_in = nc.dram_tensor("idx1", [128, idxcols], i16, kind="ExternalInput")
    idx2_in = nc.dram_tensor("idx2", [128, idxcols], i16, kind="ExternalInput")
    wfull1_in = nc.dram_tensor("wfull1", [F_IN, 132], f16, kind="ExternalInput")
    wfull2_in = nc.dram_tensor("wfull2", [HC, 132], f16, kind="ExternalInput")
    wc_in = nc.dram_tensor("wc", [HC, NCLS], f16, kind="ExternalInput")
    b1_in = nc.dram_tensor("b1", [1, HC], f32, kind="ExternalInput")
    b2_in = nc.dram_tensor("b2", [1, HC], f32, kind="ExternalInput")
    bc_in = nc.dram_tensor("bc", [1, NCLS], f32, kind="ExternalInput")
    dummy_in = nc.dram_tensor("dummyrow", [1, 128], u16, kind="ExternalInput")
    dummy8_in = nc.dram_tensor("dummyrow8", [1, 128], u8, kind="ExternalInput")
    ident_in = nc.dram_tensor("ident16", [P, P], f16, kind="ExternalInput")

    logits_out = nc.dram_tensor("logits", [npc, NCLS], f32, kind="ExternalOutput")

    t_full1 = nc.dram_tensor("t_full1", [table_rows, 128], u16)
    agb = st["ag_bounds"]
    NCH = len(agb) - 1
    t2loc = [
        nc.dram_tensor(f"t2loc{k}", [(int(agb[k + 1]) - int(agb[k])) * P, 128],
                       u16)
        for k in range(NCH)
    ]
    cutA = st["cut"]
    t2ag = nc.dram_tensor("t2ag", [table_rows, 128], u16,
                          addr_space="Shared")

    rg = [list(range(NCORES))]

    with tile.TileContext(nc) as tc:
        with (
            tc.tile_pool(name="const", bufs=1) as constp,
            tc.tile_pool(name="xt", bufs=3) as xtp,
            tc.tile_pool(name="tstage", bufs=4) as tsp,
            tc.tile_pool(name="idx", bufs=2) as idxp,
            tc.tile_pool(name="ald", bufs=1) as aldp,
            tc.tile_pool(name="g", bufs=5) as gp,
            tc.tile_pool(name="m", bufs=3) as mp,
            tc.tile_pool(name="w32", bufs=6) as wp,
            tc.tile_pool(name="norm", bufs=6) as normp,
            tc.tile_pool(name="small", bufs=6) as smallp,
            tc.tile_pool(name="hkeep", bufs=1) as hkp,
            tc.tile_pool(name="cls", bufs=1) as clsp,
            tc.tile_pool(name="psd", bufs=2, space="PSUM") as psd,
            tc.tile_pool(name="pse", bufs=3, space="PSUM") as pse,
            tc.tile_pool(name="pst", bufs=2, space="PSUM") as pstp,
            tc.tile_pool(name="psc", bufs=1, space="PSUM") as pscp,
        ):
            ACT = mybir.ActivationFunctionType
            ident = constp.tile([P, P], f16)
            nc.sync.dma_start(out=ident[:, :], in_=ident_in[:, :])
            wf1 = constp.tile([F_IN, 132], f16)
            nc.sync.dma_start(out=wf1[:, :], in_=wfull1_in[:, :])
            wf2 = constp.tile([HC, 132], f16)
            nc.sync.dma_start(out=wf2[:, :], in_=wfull2_in[:, :])
            rot1 = constp.tile([HC, HC], f16, name="rot1", tag="rot1")
            nc.sync.dma_start(out=rot1[:, :], in_=rot1_in[:, :])
            rot2 = constp.tile([HC, HC], f16, name="rot2", tag="rot2")
            nc.sync.dma_start(out=rot2[:, :], in_=rot2_in[:, :])
            wc_sb = constp.tile([HC, NCLS], f16)
            nc.sync.dma_start(out=wc_sb[:, :], in_=wc_in[:, :])
            dummy_sb = constp.tile([1, 128], u16)
            nc.sync.dma_start(out=dummy_sb[:, :], in_=dummy_in[:, :])
            dummy8_sb = constp.tile([1, 128], u8)
            nc.sync.dma_start(out=dummy8_sb[:, :], in_=dummy8_in[:, :])
            b_sb = {}
            for name, hnd, w in (("b1", b1_in, HC), ("b2", b2_in, HC),
                                 ("bc", bc_in, NCLS)):
                t = constp.tile([P, w], f32, name=f"bsb_{name}", tag=f"bsb_{name}")
                nc.sync.dma_start(out=t[:, :], in_=hnd[0:1, :].to_broadcast((P, w)))
                b_sb[name] = t
            bf16_1 = constp.tile([P, HC], f16, name="b1f16", tag="b1f16")
            nc.vector.tensor_copy(out=bf16_1[:, :], in_=b_sb["b1"][:, :])
            bf16_2 = constp.tile([P, HC], f16, name="b2f16", tag="b2f16")
            nc.vector.tensor_copy(out=bf16_2[:, :], in_=b_sb["b2"][:, :])

            # dummy rows: t_full1 both ends; A row 0; B last row
            nc.sync.dma_start(out=t_full1[0:1, :], in_=dummy_sb[:, :])
            nc.sync.dma_start(
                out=t_full1[table_rows - 1:table_rows, :], in_=dummy_sb[:, :]
            )
            nc.sync.dma_start(out=t2ag[0:1, :], in_=dummy_sb[:, :])
            nc.sync.dma_start(
                out=t2ag[table_rows - 1:table_rows, :], in_=dummy_sb[:, :]
            )

            # ---- replicated dense phase for layer 1 ----
            ald1 = aldp.tile([P, tpc, 4], f32, name="ald1", tag="ald1")
            ald2 = aldp.tile([P, tpc, 4], f32, name="ald2", tag="ald2")
            BT = 3   # tiles per PSUM batch
            GB = 12  # tiles per DMA group
            for gg in range(0, n_tiles, GB):
                gcnt = min(GB, n_tiles - gg)
                xt = xtp.tile([P, gcnt * P], f16, tag="xt")
                nc.sync.dma_start(
                    out=xt[:, :], in_=xT_in[:, gg * P:(gg + gcnt) * P]
                )
                ts_ = tsp.tile([P, gcnt, 128], u16, tag="ts")
                for g0 in range(gg, gg + gcnt, BT):
                    gn = min(BT, gg + gcnt - g0)
                    o = g0 - gg
                    ps = psd.tile([P, gn, 132], f32, tag="psd")
                    for j in range(gn):
                        nc.tensor.matmul(
                            out=ps[:, j, :],
                            lhsT=xt[:, (o + j) * P:(o + j + 1) * P],
                            rhs=wf1[:, :],
                            start=True, stop=True,
                        )
                    if (g0 // BT) % 2:  # alternate Act/DVE to halve Act load
                        nc.vector.tensor_copy(
                            out=ts_[:, o:o + gn, 0:128].bitcast(f16),
                            in_=ps[:, :, 0:128],
                        )
                    else:
                        nc.scalar.activation(
                            out=ts_[:, o:o + gn, 0:128].bitcast(f16),
                            in_=ps[:, :, 0:128],
                            func=ACT.Copy,
                        )
                    if g0 < tpc:  # own tiles: extract a_dst . h
                        jn = min(gn, tpc - g0)
                        nc.vector.tensor_copy(
                            out=ald1[:, g0:g0 + jn, :], in_=ps[:, 0:jn, 128:132]
                        )
                nc.sync.dma_start(
                    out=t_full1[1:1 + npad, :].rearrange(
                        "(p g) w -> p g w", g=n_tiles)[:, gg:gg + gcnt, :],
                    in_=ts_[:, :, :],
                )

            # ald for layer 2 is produced during layer-1 normalization.
            accA = hkp.tile([P, tpc, 132], f16, name="accA", tag="accA")

            zsAll = clsp.tile([P, tpc, NCLS], f32, name="zsAll", tag="zsAll")
            smAll = clsp.tile([P, tpc], f32, name="smAll", tag="smAll")
            lnAll = clsp.tile([P, tpc], f32, name="lnAll", tag="lnAll")
            t2b_hold = {}

            for layer in (1, 2):
                KL, KH = KLs[layer], KHs[layer]
                cumKL = np.concatenate([[0], np.cumsum(KL)])
                cumKH = np.concatenate([[0], np.cumsum(KH)])
                idx_sb = idxp.tile([128, idxcols], i16, tag="idx")
                nc.sync.dma_start(
                    out=idx_sb[:, :],
                    in_=(idx1_in if layer == 1 else idx2_in)[:, :],
                )
                t_full = t_full1
                ald = ald1 if layer == 1 else ald2
                bias = bf16_1 if layer == 1 else bf16_2
                bzero = b1_zero if layer == 1 else b2_zero

                if layer == 1:
                    KLm, KHm = KL, KH
                    cumKLm = cumKL
                    lo_base = 0
                    lo_win = t_full[0:min(WIN, table_rows), :]
                    tdt, gdt = u16, f16
                else:
                    KLm, KHm = KH, np.zeros_like(KH)  # B stream only
                    cumKLm = cumKH
                    lo_base = locol[layer]
                    lo_win = t2ag[hi_base:table_rows, :]
                    tdt, gdt = u16, f16
                    nc.vector.memset(accA[:, :, :], 0.0)
                # sub-phase A (layer 2): sources below `cut`, gathered from
                # t2ag[0:cut) as soon as all but the last AG chunk land
                def emit_a_early(r):
                    ka = int(KL[r])
                    gA = gp.tile([P, ka, 128], u16, tag="g")
                    off = 0
                    col = 8 * int(cumKL[r])
                    while off < ka:
                        jn = min(MAXCH, ka - off)
                        nc.gpsimd.dma_gather(
                            out_ap=gA[:, off:off + jn, :],
                            in_ap=t2ag[0:cutA, :],
                            idxs_ap=idx_sb[:, col:col + jn * 8],
                            num_idxs=jn * P,
                            num_idxs_reg=jn * P,
                            elem_size=128,
                        )
                        off += jn
                        col += jn * 8
                    wA = wp.tile([P, ka, 4], f32, tag="w32")
                    nc.vector.tensor_tensor(
                        out=wA[:, :, :],
                        in0=gA[:, :, :].bitcast(f16).rearrange(
                            "p k (h c) -> p k h c", h=H)[:, :, :, 0],
                        in1=ald[:, r:r + 1, :].to_broadcast((P, ka, 4)),
                        op=mybir.AluOpType.add,
                    )
                    zA = wp.tile([P, ka, 4], f32, tag="zt")
                    nc.scalar.activation(
                        out=zA[:, :, :], in_=wA[:, :, :],
                        func=ACT.Prelu, alpha=NEG,
                    )
                    mA = mp.tile([P, ka, 132], f16, tag="m")
                    nc.scalar.activation(
                        out=mA[:, :, 128:132], in_=zA[:, :, :], func=ACT.Exp,
                    )
                    aA = wp.tile([P, ka, 4, 2], f16, tag="a2")
                    nc.vector.tensor_copy(
                        out=aA[:, :, :, :],
                        in_=mA[:, :, 128:132][:, :, :, None].to_broadcast(
                            (P, ka, 4, 2)),
                    )
                    nc.vector.tensor_tensor(
                        out=mA[:, :, 0:128].rearrange(
                            "p k (h c d) -> p k h c d", h=H, d=2),
                        in0=gA[:, :, :].bitcast(f16).rearrange(
                            "p k (h c d) -> p k h c d", h=H, d=2),
                        in1=aA[:, :, :, None, :].to_broadcast(
                            (P, ka, 4, 16, 2)),
                        op=mybir.AluOpType.mult,
                    )
                    psA = pse.tile([P, 2, 132], f32, tag="pse")
                    npairs = ka // 2
                    for j in range(npairs):
                        nc.tensor.matmul(
                            out=psA[:, :, :], lhsT=ident[:, :],
                            rhs=mA[:, 2 * j:2 * j + 2, :],
                            start=(j == 0),
                            stop=(j == npairs - 1 and ka % 2 == 0),
                        )
                    if ka % 2:
                        nc.tensor.matmul(
                            out=psA[:, 0, :], lhsT=ident[:, :],
                            rhs=mA[:, ka - 1, :],
                            start=(npairs == 0), stop=True,
                        )
                    return psA

                def emit_a_late(r, psA):
                    accA0 = normp.tile([P, 132], f16, tag="acc0")
                    nc.scalar.activation(
                        out=accA0[:, :], in_=psA[:, 0, :], func=ACT.Copy,
                    )
                    nc.vector.tensor_tensor(
                        out=accA[:, r, :], in0=accA0[:, :], in1=psA[:, 1, :],
                        op=mybir.AluOpType.add,
                    )

                if layer == 2:
                    apend = []
                    for r in range(tpc):
                        if int(KL[r]) == 0:
                            continue
                        psA_r = emit_a_early(r)
                        apend.append((r, psA_r))
                        if len(apend) > 1:
                            emit_a_late(*apend.pop(0))
                    while apend:
                        emit_a_late(*apend.pop(0))
                # main per-round loop (layer 1: lo+hi; layer 2: B stream),
                # software-pipelined: round r+1's gather/mult half is emitted
                # before round r's normalize half so in-order engine queues
                # don't serialize the cross-engine tail chain between rounds.
                def emit_early(r):
                    kl, kh = int(KLm[r]), int(KHm[r])
                    K = kl + kh
                    g = gp.tile([P, K, 128], tdt, tag="g")
                    # lo gather
                    off = 0
                    col = lo_base + 8 * int(cumKLm[r])
                    while off < kl:
                        jn = min(MAXCH, kl - off)
                        nc.gpsimd.dma_gather(
                            out_ap=g[:, off:off + jn, :],
                            in_ap=lo_win,
                            idxs_ap=idx_sb[:, col:col + jn * 8],
                            num_idxs=jn * P,
                            num_idxs_reg=jn * P,
                            elem_size=128,
                        )
                        off += jn
                        col += jn * 8
                    # hi gather
                    off = 0
                    col = locol[layer] + 8 * int(cumKH[r])
                    while off < kh:
                        jn = min(MAXCH, kh - off)
                        nc.gpsimd.dma_gather(
                            out_ap=g[:, kl + off:kl + off + jn, :],
                            in_ap=t_full[hi_base:table_rows, :],
                            idxs_ap=idx_sb[:, col:col + jn * 8],
                            num_idxs=jn * P,
                            num_idxs_reg=jn * P,
                            elem_size=128,
                        )
                        off += jn
                        col += jn * 8

                    # w = exp(prelu(als_src + ald_dst))
                    w32 = wp.tile([P, K, 4], f32, tag="w32")
                    nc.vector.tensor_tensor(
                        out=w32[:, :, :],
                        in0=g[:, :, :].bitcast(gdt).rearrange(
                            "p k (h c) -> p k h c", h=H)[:, :, :, 0],
                        in1=ald[:, r:r + 1, :].to_broadcast((P, K, 4)),
                        op=mybir.AluOpType.add,
                    )
                    zt = wp.tile([P, K, 4], f32, tag="zt")
                    if NO_PRELU:
                        nc.vector.tensor_scalar_mul(
                            out=zt[:, :, :], in0=w32[:, :, :], scalar1=NEG,
                        )
                        nc.vector.tensor_tensor(
                            out=zt[:, :, :], in0=w32[:, :, :], in1=zt[:, :, :],
                            op=mybir.AluOpType.max,
                        )
                    else:
                        nc.scalar.activation(
                            out=zt[:, :, :], in_=w32[:, :, :],
                            func=ACT.Prelu, alpha=NEG,
                        )
                    m = mp.tile([P, K, 132], f16, tag="m")
                    nc.scalar.activation(
                        out=m[:, :, 128:132], in_=zt[:, :, :], func=ACT.Exp,
                    )
                    a2 = wp.tile([P, K, 4, 2], f16, tag="a2")
                    nc.vector.tensor_copy(
                        out=a2[:, :, :, :],
                        in_=m[:, :, 128:132][:, :, :, None].to_broadcast(
                            (P, K, 4, 2)),
                    )
                    nc.vector.tensor_tensor(
                        out=m[:, :, 0:128].rearrange(
                            "p k (h c d) -> p k h c d", h=H, d=2),
                        in0=g[:, :, :].bitcast(gdt).rearrange(
                            "p k (h c d) -> p k h c d", h=H, d=2),
                        in1=a2[:, :, :, None, :].to_broadcast((P, K, 4, 16, 2)),
                        op=mybir.AluOpType.mult,
                    )
                    # accumulate the round in PSUM via identity matmuls,
                    # two chunks per matmul (halves PE instruction count)
                    ps = pse.tile([P, 2, 132], f32, tag="pse")
                    if NO_PAIR:
                        nc.vector.memset(ps[:, 1, :], 0.0)
                        for j in range(K):
                            nc.tensor.matmul(
                                out=ps[:, 0, :], lhsT=ident[:, :], rhs=m[:, j, :],
                                start=(j == 0), stop=(j == K - 1),
                            )
                    else:
                        npairs = K // 2
                        for j in range(npairs):
                            nc.tensor.matmul(
                                out=ps[:, :, :], lhsT=ident[:, :],
                                rhs=m[:, 2 * j:2 * j + 2, :],
                                start=(j == 0),
                                stop=(j == npairs - 1 and K % 2 == 0),
                            )
                        if K % 2:
                            nc.tensor.matmul(
                                out=ps[:, 0, :], lhsT=ident[:, :],
                                rhs=m[:, K - 1, :],
                                start=False, stop=True,
                            )
                    acc0 = normp.tile([P, 132], f16, tag="acc0")
                    nc.scalar.activation(
                        out=acc0[:, :], in_=ps[:, 0, :], func=ACT.Copy,
                    )
                    accf = normp.tile([P, 132], f16, tag="accf")
                    nc.vector.tensor_tensor(
                        out=accf[:, :], in0=acc0[:, :], in1=ps[:, 1, :],
                        op=mybir.AluOpType.add,
                    )
                    if layer == 2:
                        accfB = normp.tile([P, 132], f16, tag="accfB")
                        nc.vector.tensor_tensor(
                            out=accfB[:, :], in0=accf[:, :],
                            in1=accA[:, r, :],
                            op=mybir.AluOpType.add,
                        )
                        accf = accfB
                    return accf

                def emit_late(r, accf):
                    # normalize + bias + leaky
                    den = smallp.tile([P, 4], f16, tag="den")
                    nc.vector.tensor_scalar_add(
                        out=den[:, :], in0=accf[:, 128:132], scalar1=1e-4,
                    )
                    rc2 = smallp.tile([P, 4, 2], f16, tag="rc2")
                    with nc.allow_low_precision(reason="f16 softmax denom"):
                        nc.vector.reciprocal(out=rc2[:, :, 0], in_=den[:, :])
                    nc.vector.tensor_copy(
                        out=rc2[:, :, 1], in_=rc2[:, :, 0],
                    )
                    xn = normp.tile([P, HC], f16, tag="xn")
                    nc.vector.tensor_tensor(
                        out=xn[:, :].rearrange("p (h c d) -> p h c d", h=H, d=2),
                        in0=accf[:, 0:128].rearrange(
                            "p (h c d) -> p h c d", h=H, d=2),
                        in1=rc2[:, :, None, :].to_broadcast((P, 4, 16, 2)),
                        op=mybir.AluOpType.mult,
                    )
                    ptry = pstp.tile([P, P], f16, tag="pst")
                    nc.tensor.transpose(
                        out=ptry[:, :], in_=xn[:, :], identity=ident[:, :]
                    )
                    yT = normp.tile([P, P], f16, tag="yT")
                    nc.scalar.activation(
                        out=yT[:, :], in_=ptry[:, :], func=ACT.Copy,
                    )
                    prot = pstp.tile([P, HC], f32, tag="pst")
                    nc.tensor.matmul(
                        out=prot[:, :], lhsT=yT[:, :],
                        rhs=(rot1 if layer == 1 else rot2)[:, :],
                        start=True, stop=True,
                    )
                    xh = normp.tile([P, HC], f16, tag="xh")
                    nc.scalar.activation(
                        out=xh[:, :], in_=prot[:, :], func=ACT.Copy,
                    )
                    if not bzero:
                        nc.vector.tensor_tensor(
                            out=xh[:, :], in0=xh[:, :], in1=bias[:, :],
                            op=mybir.AluOpType.add,
                        )
                    hnext = normp.tile([P, HC], f16, tag="hnext")
                    if NO_PRELU:
                        nc.vector.tensor_scalar_mul(
                            out=hnext[:, :], in0=xh[:, :], scalar1=NEG,
                        )
                        nc.vector.tensor_tensor(
                            out=hnext[:, :], in0=xh[:, :], in1=hnext[:, :],
                            op=mybir.AluOpType.max,
                        )
                    else:
                        nc.scalar.activation(
                            out=hnext[:, :], in_=xh[:, :], func=ACT.Prelu,
                            alpha=NEG,
                        )

                    ptr = pstp.tile([P, P], f16, tag="pst")
                    nc.tensor.transpose(
                        out=ptr[:, :], in_=hnext[:, :], identity=ident[:, :]
                    )
                    hT = normp.tile([P, P], f16, tag="hT")
                    nc.scalar.activation(
                        out=hT[:, :], in_=ptr[:, :], func=ACT.Copy,
                    )
                    if layer == 1:
                        ps2 = psd.tile([P, 1, 132], f32, tag="psd")
                        nc.tensor.matmul(
                            out=ps2[:, 0, :], lhsT=hT[:, :], rhs=wf2[:, :],
                            start=True, stop=True,
                        )
                        kch = int(np.searchsorted(agb, r, side="right")) - 1
                        rr = r - int(agb[kch])
                        tk = int(agb[kch + 1]) - int(agb[kch])
                        # tiles per t2loc store (contig per partition); the
                        # first chunk flushes eagerly so AG0 fires sooner
                        SB = 2 if kch == 0 else 4
                        if rr % SB == 0:
                            t2b = tsp.tile(
                                [P, min(SB, tk - rr), 128], u16,
                                name="t2b", tag="t2s")
                            t2b_hold[0] = t2b
                        t2b = t2b_hold[0]
                        nc.scalar.activation(
                            out=t2b[:, rr % SB, :].bitcast(f16),
                            in_=ps2[:, 0, 0:128], func=ACT.Copy,
                        )
                        nc.vector.tensor_copy(
                            out=ald2[:, r, :], in_=ps2[:, 0, 128:132]
                        )
                        if rr % SB == SB - 1 or rr == tk - 1:
                            cnt = rr % SB + 1
                            r0 = rr - cnt + 1
                            nc.sync.dma_start(
                                out=t2loc[kch].rearrange(
                                    "(p t) w -> p t w", t=tk)[:, r0:r0 + cnt, :],
                                in_=t2b[:, 0:cnt, :],
                            )
                        if r == int(agb[kch + 1]) - 1:
                            nrows = tk * P
                            base = 1 + sum(
                                (int(agb[j + 1]) - int(agb[j])) * P * NCORES
                                for j in range(kch))
                            gtot = nrows * NCORES
                            agout = t2ag[base:base + gtot, :]
                            nc.gpsimd.collective_compute(
                                "AllGather",
                                mybir.AluOpType.bypass,
                                ins=[t2loc[kch][:, :]],
                                outs=[agout],
                                replica_groups=rg,
                            )
                    else:
                        pc = pscp.tile([P, NCLS], f32, tag="pc")
                        nc.tensor.matmul(
                            out=pc[:, :], lhsT=hT[:, :], rhs=wc_sb[:, :],
                            start=True, stop=True,
                        )
                        lg = normp.tile([P, NCLS], f32, tag="lg")
                        nc.scalar.activation(
                            out=lg[:, :], in_=pc[:, :], func=ACT.Copy,
                        )
                        nc.vector.tensor_tensor(
                            out=lg[:, :], in0=lg[:, :], in1=b_sb["bc"][:, :],
                            op=mybir.AluOpType.add,
                        )
                        mx = smallp.tile([P, 1], f32, tag="mx")
                        nc.vector.reduce_max(
                            out=mx[:, :], in_=lg[:, :], axis=mybir.AxisListType.X
                        )
                        nc.vector.tensor_scalar(
                            out=zsAll[:, r, :], in0=lg[:, :], scalar1=mx[:, :],
                            scalar2=None, op0=mybir.AluOpType.subtract,
                        )
                        es = normp.tile([P, NCLS], f32, tag="lg2")
                        nc.scalar.activation(
                            out=es[:, :], in_=zsAll[:, r, :], func=ACT.Exp,
                        )
                        nc.vector.reduce_sum(
                            out=smAll[:, r:r + 1], in_=es[:, :],
                            axis=mybir.AxisListType.X,
                        )

                DEPTH = 1
                pend = []
                for r in range(tpc):
                    acc_r = emit_early(r)
                    pend.append((r, acc_r))
                    if len(pend) > DEPTH:
                        emit_late(*pend.pop(0))
                while pend:
                    emit_late(*pend.pop(0))

            # ---- batched log-softmax tail ----
            nc.scalar.activation(
                out=lnAll[:, :], in_=smAll[:, :], func=ACT.Ln,
            )
            otAll = clsp.tile([P, tpc, NCLS], f32, name="otAll", tag="otAll")
            for r in range(tpc):
                nc.vector.tensor_scalar(
                    out=otAll[:, r, :], in0=zsAll[:, r, :],
                    scalar1=lnAll[:, r:r + 1], scalar2=None,
                    op0=mybir.AluOpType.subtract,
                )
            nc.sync.dma_start(
                out=logits_out[:, :].rearrange("(l r) c -> l r c", r=tpc),
                in_=otAll[:, :, :],
            )

    nc.finalize()
    return nc


# ---------------------------------------------------------------------------
# entry point
# ---------------------------------------------------------------------------

_CACHE = {}


def kernel(x, edge_index, W1, a1_src, a1_dst, b1, W2, a2_src, a2_dst, b2, Wc, bc):
    global LAST_EXEC_NS
    import os

    x = np.asarray(x, dtype=np.float32)
    n_real = x.shape[0]
    b1 = np.asarray(b1, dtype=np.float32)
    b2 = np.asarray(b2, dtype=np.float32)
    b1_zero = bool(np.all(b1 == 0))
    b2_zero = bool(np.all(b2 == 0))
    ekey = hash(np.asarray(edge_index).tobytes()) ^ hash((n_real, b1_zero, b2_zero))
    if ekey in _CACHE:
        nc, st = _CACHE[ekey]
    else:
        st = _preprocess(x, edge_index, n_real)
        nc = _build(st, b1_zero, b2_zero)
        _CACHE[ekey] = (nc, st)

    npad, npc, tpc = st["npad"], st["npc"], st["tpc"]
    new_id = st["new_id"]

    # per-core rotated xT (full table, f16)
    x_pad = np.zeros((npad, F_IN), dtype=np.float32)
    x_pad[new_id[:n_real]] = x
    x_blocks = x_pad.reshape(NCORES, npc, F_IN)

    wfull1, rot1 = _wfull(W1, a1_src, a1_dst)
    wfull2, rot2 = _wfull(W2, a2_src, a2_dst)
    wc = np.ascontiguousarray(np.asarray(Wc, dtype=np.float16))
    b1r = b1[None, :]
    b2r = b2[None, :]
    bcr = np.asarray(bc, dtype=np.float32)[None, :]
    dummy = _dummy_row()
    dummy8 = _dummy_row8()
    ident = np.eye(P, dtype=np.float16)

    idxcols = max(st["idx1"].shape[2], st["idx2"].shape[2], 16)

    def pad_idx(a):
        if a.shape[2] < idxcols:
            a = np.concatenate(
                [a, np.zeros((NCORES, 128, idxcols - a.shape[2]), np.int16)],
                axis=2)
        return a

    idx1 = pad_idx(st["idx1"])
    idx2 = pad_idx(st["idx2"])

    in_maps = []
    for c in range(NCORES):
        rot = np.roll(np.arange(NCORES), -c)  # own block first
        xT = np.ascontiguousarray(
            x_blocks[rot].reshape(npad, F_IN).T.astype(np.float16)
        )
        in_maps.append({
            "xT": xT,
            "idx1": np.ascontiguousarray(idx1[c]),
            "idx2": np.ascontiguousarray(idx2[c]),
            "wfull1": wfull1, "wfull2": wfull2, "wc": wc,
            "rot1": rot1, "rot2": rot2,
            "b1": b1r, "b2": b2r, "bc": bcr,
            "dummyrow": dummy, "dummyrow8": dummy8, "ident16": ident,
        })

    os.environ.setdefault("BASS_NEVER_TRACE", "1")
    res = run_bass_kernel_spmd(nc, in_maps, core_ids=list(range(NCORES)))
    LAST_EXEC_NS = res.exec_time_ns

    # device rows are p-major (row = lane*tpc + r); restore node order
    logits_pad = np.concatenate(
        [res.results[c]["logits"].reshape(P, tpc, NCLS)
         .transpose(1, 0, 2).reshape(npc, NCLS)
         for c in range(NCORES)], axis=0
    )
    return logits_pad[new_id[:n_real]].astype(np.float32)



# revision 68
# speedup vs baseline: 1.0114x; 1.0114x over previous
"""Trainium2 Bass kernel for 2-layer GAT node classification (50K nodes, 800K edges).

Design (vs. the gather-everything baseline):
  - Layer 1 runs with NO collective: x is a full input on every core, so each
    core computes the FULL node table locally (replicated dense phase).
  - Node features travel in a per-head rotated basis ("y-space"): an
    orthogonal Householder transform with first row = a_src/||a_src|| is
    folded into the dense weights on the host, so y[h*32] IS the attention
    source logit. Table rows shrink to 256B (the dma_gather minimum), halving
    gather traffic; messages aggregate linearly in y-space and a per-round
    128x128 rotate-back matmul restores h-space before the nonlinear
    leaky-relu.
  - The layer-2 table is distributed by a 4-chunk AllGather over round
    blocks (small first chunk), overlapping the collective with the layer-1
    edge phase; the chunk-permuted row layout is baked into the host-built
    gather indices.
  - Per-core node order is rotated (own block first) so the SPMD program
    extracts own-destination data at fixed positions.
  - Edge phase is per-destination-round: dma_gathers per (round, window),
    per-round PSUM accumulation via paired identity matmuls, leaky/exp on
    the Activation engine (Prelu+Exp+Ln+Copy share one act table), alpha
    duplicated into pairs so the message multiply hits the DVE 2x mode,
    normalization fused per round, log-softmax Ln batched across rounds.
"""
import sys

sys.path.insert(0, "/opt/trn_rl_repo")

import numpy as np

import concourse.bacc as bacc
import concourse.tile as tile
import concourse.mybir as mybir
from concourse.bass_utils import run_bass_kernel_spmd

P = 128
NCORES = 8
F_IN = 128
H = 4
C = 32
HC = 128
NCLS = 40
NEG = 0.2
WIN = 32768
RING = 16384  # SWDGE ring; >1024 idxs per call hangs the device
MAXCH = (RING // 16) // P  # max chunks per dma_gather call

f32 = mybir.dt.float32
f16 = mybir.dt.float16
u16 = mybir.dt.uint16
i16 = mybir.dt.int16
u8 = mybir.dt.uint8
f8 = mybir.dt.float8e4  # e4m3

LAST_EXEC_NS = None
import os as _os
NO_PRELU = _os.environ.get("V2_NO_PRELU", "0") == "1"
NO_PAIR = _os.environ.get("V2_NO_PAIR", "0") == "1"
NO_LN = _os.environ.get("V2_NO_LN", "0") == "1"
NO_GATHER = _os.environ.get("V2_NO_GATHER", "0") == "1"
NO_COLL = _os.environ.get("V2_NO_COLL", "0") == "1"



# ---------------------------------------------------------------------------
# host preprocessing
# ---------------------------------------------------------------------------

def _cumcount(keys):
    n = len(keys)
    if n == 0:
        return np.zeros(0, dtype=np.int64)
    first = np.ones(n, dtype=bool)
    first[1:] = keys[1:] != keys[:-1]
    idx = np.arange(n)
    start = np.maximum.accumulate(np.where(first, idx, 0))
    return idx - start


def _build_grids(src_row, dst_newid, npc, tpc, table_rows,
                 lo_cut=None, lo_frac=0.5):
    """Per-core slot grids for one layer.

    src_row: [Etot, NCORES] table row of the source as seen by each core
             (layer 1: rotated; layer 2: same global row for all cores).
    dst_newid: [Etot] global new id of the destination.
    lo_cut: rows >= lo_cut are hi-only even if < WIN (layer 2: rows of the
            last AG chunk are not yet valid when the lo/A stream gathers).
    lo_frac: target fraction of a node's edges assigned to the lo stream.
    Returns KL, KH [tpc] (common across cores) and per-core packed slot
    arrays (values = window-relative table rows).
    """
    hi_base = max(0, table_rows - WIN)
    if lo_cut is None:
        lo_cut = WIN
    dst_core = dst_newid // npc
    r_e = (dst_newid % npc) // P
    lane_e = dst_newid % P

    kl_counts = np.zeros((NCORES, tpc, P), dtype=np.int64)
    kh_counts = np.zeros((NCORES, tpc, P), dtype=np.int64)
    per_core = []
    for c in range(NCORES):
        m = dst_core == c
        rows = src_row[m, c] if src_row.ndim == 2 else src_row[m]
        d_r = r_e[m]
        d_lane = lane_e[m]
        cat = np.full(len(rows), 2, dtype=np.int8)  # flex
        cat[rows < hi_base] = 0  # lo only
        cat[rows >= lo_cut] = 1  # hi only
        dkey = d_r * P + d_lane
        o = np.argsort(dkey, kind="stable")
        rows, d_r, d_lane, cat, dkey = rows[o], d_r[o], d_lane[o], cat[o], dkey[o]
        ndeg = np.bincount(dkey, minlength=tpc * P)
        nlo = np.bincount(dkey[cat == 0], minlength=tpc * P)
        nhi = np.bincount(dkey[cat == 1], minlength=tpc * P)
        tgt = (ndeg * lo_frac + 0.5).astype(np.int64)
        kl_node = np.maximum(nlo, np.minimum(ndeg - nhi, tgt))
        flex_rank = np.zeros(len(rows), dtype=np.int64)
        mflex = cat == 2
        flex_rank[mflex] = _cumcount(dkey[mflex])
        to_lo = (cat == 0) | (mflex & (flex_rank < (kl_node - nlo)[dkey]))
        k_slot = np.zeros(len(rows), dtype=np.int64)
        for mm in (to_lo, ~to_lo):
            k_slot[mm] = _cumcount(dkey[mm])
        kl_counts[c] = kl_node.reshape(tpc, P)
        kh_counts[c] = (ndeg - kl_node).reshape(tpc, P)
        per_core.append((rows, d_r, d_lane, to_lo, k_slot))

    KL = kl_counts.max(axis=(0, 2)).astype(np.int64)
    KH = kh_counts.max(axis=(0, 2)).astype(np.int64)
    cumKL = np.concatenate([[0], np.cumsum(KL)])
    cumKH = np.concatenate([[0], np.cumsum(KH)])
    CL, CH = int(cumKL[-1]), int(cumKH[-1])

    DUM_LO = 0
    DUM_HI = table_rows - 1 - hi_base
    slots_lo = np.full((NCORES, CL * P), DUM_LO, dtype=np.int64)
    slots_hi = np.full((NCORES, CH * P), DUM_HI, dtype=np.int64)
    for c in range(NCORES):
        rows, d_r, d_lane, to_lo, k_slot = per_core[c]
        pos_lo = (cumKL[d_r] + k_slot) * P + d_lane
        pos_hi = (cumKH[d_r] + k_slot) * P + d_lane
        slots_lo[c, pos_lo[to_lo]] = rows[to_lo]
        slots_hi[c, pos_hi[~to_lo]] = rows[~to_lo] - hi_base
    return KL, KH, slots_lo, slots_hi


def _build_grids_fixed(src_row, dst_newid, npc, tpc, cut, dum_b):
    """Two-category grid with FIXED assignment: A = row < cut, B = else.
    Slot values: A-stream = row; B-stream = row - cut."""
    dst_core = dst_newid // npc
    r_e = (dst_newid % npc) // P
    lane_e = dst_newid % P
    ka_c = np.zeros((NCORES, tpc, P), dtype=np.int64)
    kb_c = np.zeros((NCORES, tpc, P), dtype=np.int64)
    per_core = []
    for c in range(NCORES):
        m = dst_core == c
        rows = src_row[m]
        d_r = r_e[m]
        d_lane = lane_e[m]
        dkey = d_r * P + d_lane
        o = np.argsort(dkey, kind="stable")
        rows, dkey = rows[o], dkey[o]
        isA = rows < cut
        ka_c[c] = np.bincount(dkey[isA], minlength=tpc * P).reshape(tpc, P)
        kb_c[c] = np.bincount(dkey[~isA], minlength=tpc * P).reshape(tpc, P)
        k_slot = np.zeros(len(rows), dtype=np.int64)
        for mm in (isA, ~isA):
            k_slot[mm] = _cumcount(dkey[mm])
        per_core.append((rows, dkey, isA, k_slot))
    KA = ka_c.max(axis=(0, 2)).astype(np.int64)
    KB = kb_c.max(axis=(0, 2)).astype(np.int64)
    cumKA = np.concatenate([[0], np.cumsum(KA)])
    cumKB = np.concatenate([[0], np.cumsum(KB)])
    slots_a = np.full((NCORES, int(cumKA[-1]) * P), 0, dtype=np.int64)
    slots_b = np.full((NCORES, int(cumKB[-1]) * P), dum_b, dtype=np.int64)
    for c in range(NCORES):
        rows, dkey, isA, k_slot = per_core[c]
        d_r = dkey // P
        lane = dkey % P
        pa = (cumKA[d_r] + k_slot) * P + lane
        pb = (cumKB[d_r] + k_slot) * P + lane
        slots_a[c, pa[isA]] = rows[isA]
        slots_b[c, pb[~isA]] = rows[~isA] - cut
    return KA, KB, slots_a, slots_b


def _pack(slots):
    """[NCORES, n_slots] -> [NCORES, 128, n_slots//16] int16 idx layout."""
    ncols = slots.shape[1] // 16
    if ncols == 0:
        return np.zeros((NCORES, 128, 0), np.int16)
    a = slots.reshape(NCORES, ncols, 16).transpose(0, 2, 1)
    a = a.astype(np.uint16).view(np.int16)
    return np.tile(a, (1, 8, 1))


def _preprocess(x, edge_index, n_real):
    n_tiles = -(-(n_real + 1) // P)
    n_tiles = -(-n_tiles // NCORES) * NCORES
    npad = n_tiles * P
    tpc = n_tiles // NCORES
    npc = tpc * P
    # partition-major table: row = 1 + lane*n_tiles + gtile (dummy rows at
    # both ends).  A multi-tile store is then contiguous per partition.
    table_rows = 1 + npad + 1
    assert table_rows <= 2 * WIN, "two int16 windows must cover the table"

    src0 = np.asarray(edge_index[0]).astype(np.int64)
    dst0 = np.asarray(edge_index[1]).astype(np.int64)

    deg = np.bincount(dst0, minlength=npad).astype(np.int64)
    deg[:n_real] += 1
    order = np.argsort(deg, kind="stable")
    pos = np.empty(npad, dtype=np.int64)
    pos[order] = np.arange(npad)
    tile_of = pos // P
    lane_of = pos % P
    r_of = tile_of // NCORES
    c_of = tile_of % NCORES
    new_id = c_of * npc + r_of * P + lane_of  # old -> global new id

    all_src = np.concatenate([new_id[src0], new_id[:n_real]])
    all_dst = np.concatenate([new_id[dst0], new_id[:n_real]])

    # layer-1 source rows: rotated per core (own block first), p-major
    blk = all_src // npc
    s_r = (all_src % npc) // P
    s_l = all_src % P
    rot_rows = np.empty((len(all_src), NCORES), dtype=np.int64)
    for c in range(NCORES):
        rot_rows[:, c] = 1 + s_l * n_tiles + ((blk - c) % NCORES) * tpc + s_r
    KL1, KH1, sl1, sh1 = _build_grids(rot_rows, all_dst, npc, tpc, table_rows)
    # layer-2 source rows: chunked-AllGather layout. The AG runs in NCH
    # chunks over round-blocks; chunk k of every core lands rank-major at
    # base_k. row(c, r, l) = 128 + base_k + c*rows_k + (r - r0_k)*128 + l.
    # chunks 0..NCH-2 must fit below WIN so the A stream (which gathers from
    # window [0, WIN) before the last AG lands) can reach them
    a_tiles = min(tpc, (WIN - 1) // (P * NCORES))
    if tpc > a_tiles:
        b0 = max(1, a_tiles // 4)
        b1 = max(b0 + 1, (a_tiles * 4) // 7)
        bounds = np.unique(np.array(
            [0, b0, b1, a_tiles, (a_tiles + tpc + 1) // 2, tpc]))
    else:
        bounds = np.unique(np.array(
            [0, max(1, tpc // 4), max(2, (4 * tpc) // 7), tpc]))
    NCH = len(bounds) - 1
    src_c = all_src // npc
    src_r = (all_src % npc) // P
    src_l = all_src % P
    chunk_of = np.searchsorted(bounds, src_r, side="right") - 1
    tiles_k = bounds[1:] - bounds[:-1]
    rows_k = tiles_k * P
    base_k = np.concatenate([[0], np.cumsum(rows_k * NCORES)])
    # p-major within each (chunk, core) block: local row = lane*tiles + rr
    l2_rows = (1 + base_k[chunk_of] + src_c * rows_k[chunk_of]
               + src_l * tiles_k[chunk_of] + (src_r - bounds[chunk_of]))
    cut = 1 + a_tiles * P * NCORES  # rows gatherable by the A stream
    assert cut <= WIN
    KL2, KH2, sl2, sh2 = _build_grids(
        l2_rows, all_dst, npc, tpc, table_rows, lo_cut=cut, lo_frac=0.62)

    idx1 = np.concatenate([_pack(sl1), _pack(sh1)], axis=2)
    idx2 = np.concatenate([_pack(sl2), _pack(sh2)], axis=2)

    return dict(
        npad=npad, npc=npc, tpc=tpc, table_rows=table_rows,
        ag_bounds=bounds, cut=cut,
        KL1=KL1, KH1=KH1, KL2=KL2, KH2=KH2,
        idx1=np.ascontiguousarray(idx1), idx2=np.ascontiguousarray(idx2),
        new_id=new_id, n_real=n_real,
    )


def _yfold(a_src):
    """Per-head transform T = D @ Q_house (y = T h, y0 = a_src . h) and its
    inverse R = T^{-1} as 128x128 block-diagonal f32 matrices."""
    a = np.asarray(a_src, np.float32)
    T = np.zeros((HC, HC), dtype=np.float64)
    R = np.zeros((HC, HC), dtype=np.float64)
    for h in range(H):
        ah = a[h].astype(np.float64)
        na = np.linalg.norm(ah)
        ahat = ah / na
        v = ahat - np.eye(C)[0]
        if np.linalg.norm(v) < 1e-12:
            Q = np.eye(C)
        else:
            v = v / np.linalg.norm(v)
            Q = np.eye(C) - 2.0 * np.outer(v, v)
        # Q is symmetric orthogonal with Q[0,:] = ahat
        Qo = Q.copy()
        Qo[0, :] = ahat  # guard sign: householder gives exactly this row
        D = np.eye(C)
        D[0, 0] = na
        Th = D @ Qo
        Rh = Qo.T @ np.diag([1.0 / na] + [1.0] * (C - 1))
        T[h * C:(h + 1) * C, h * C:(h + 1) * C] = Th
        R[h * C:(h + 1) * C, h * C:(h + 1) * C] = Rh
    return T.astype(np.float32), R.astype(np.float32)


def _wfull(W, a_src, a_dst):
    """[Wf @ T.T | Wf @ Wad] (132 cols, f16) plus rotate-back R (f16)."""
    W = np.asarray(W, dtype=np.float32)
    fin = W.shape[0]
    Wf = W.reshape(fin, HC)
    T, R = _yfold(a_src)
    Wad = np.zeros((HC, H), dtype=np.float32)
    for h in range(H):
        Wad[h * C:(h + 1) * C, h] = np.asarray(a_dst, np.float32)[h]
    out = np.concatenate([Wf @ T.T, Wf @ Wad], axis=1)  # [fin, 132]
    return (np.ascontiguousarray(out.astype(np.float16)),
            np.ascontiguousarray(R.astype(np.float16)))


def _dummy_row():
    row = np.zeros(128, dtype=np.float16)
    for h in range(H):
        row[h * C] = -60000.0
    return row.view(np.uint16)[None, :]


def _dummy_row8():
    import ml_dtypes
    row = np.zeros(128, dtype=ml_dtypes.float8_e4m3fn)
    for h in range(H):
        row[h * C] = -448.0
    return row.view(np.uint8)[None, :]


# ---------------------------------------------------------------------------
# device program
# ---------------------------------------------------------------------------

def _build(st, b1_zero, b2_zero):
    npc, tpc = st["npc"], st["tpc"]
    npad = st["npad"]
    table_rows = st["table_rows"]
    hi_base = max(0, table_rows - WIN)
    n_tiles = npad // P
    KLs = {1: st["KL1"], 2: st["KL2"]}
    KHs = {1: st["KH1"], 2: st["KH2"]}
    ncols = {1: st["idx1"].shape[2], 2: st["idx2"].shape[2]}
    locol = {
        1: 8 * int(st["KL1"].sum()),
        2: 8 * int(st["KL2"].sum()),
    }
    Kmax = max(
        int((st["KL1"] + st["KH1"]).max()), int((st["KL2"] + st["KH2"]).max())
    )
    idxcols = max(ncols[1], ncols[2], 16)

    nc = bacc.Bacc(None, target_bir_lowering=False,
                   dynamic_dma_scratch_size=RING)

    xT_in = nc.dram_tensor("xT", [F_IN, npad], f16, kind="ExternalInput")
    rot1_in = nc.dram_tensor("rot1", [HC, HC], f16, kind="ExternalInput")
    rot2_in = nc.dram_tensor("rot2", [HC, HC], f16, kind="ExternalInput")
    idx1_in = nc.dram_tensor("idx1", [128, idxcols], i16, kind="ExternalInput")
    idx2_in = nc.dram_tensor("idx2", [128, idxcols], i16, kind="ExternalInput")
    wfull1_in = nc.dram_tensor("wfull1", [F_IN, 132], f16, kind="ExternalInput")
    wfull2_in = nc.dram_tensor("wfull2", [HC, 132], f16, kind="ExternalInput")
    wc_in = nc.dram_tensor("wc", [HC, NCLS], f16, kind="ExternalInput")
    b1_in = nc.dram_tensor("b1", [1, HC], f32, kind="ExternalInput")
    b2_in = nc.dram_tensor("b2", [1, HC], f32, kind="ExternalInput")
    bc_in = nc.dram_tensor("bc", [1, NCLS], f32, kind="ExternalInput")
    dummy_in = nc.dram_tensor("dummyrow", [1, 128], u16, kind="ExternalInput")
    dummy8_in = nc.dram_tensor("dummyrow8", [1, 128], u8, kind="ExternalInput")
    ident_in = nc.dram_tensor("ident16", [P, P], f16, kind="ExternalInput")

    logits_out = nc.dram_tensor("logits", [npc, NCLS], f32, kind="ExternalOutput")

    t_full1 = nc.dram_tensor("t_full1", [table_rows, 128], u16)
    agb = st["ag_bounds"]
    NCH = len(agb) - 1
    t2loc = [
        nc.dram_tensor(f"t2loc{k}", [(int(agb[k + 1]) - int(agb[k])) * P, 128],
                       u16)
        for k in range(NCH)
    ]
    cutA = st["cut"]
    t2ag = nc.dram_tensor("t2ag", [table_rows, 128], u16,
                          addr_space="Shared")

    rg = [list(range(NCORES))]

    with tile.TileContext(nc) as tc:
        with (
            tc.tile_pool(name="const", bufs=1) as constp,
            tc.tile_pool(name="xt", bufs=3) as xtp,
            tc.tile_pool(name="tstage", bufs=4) as tsp,
            tc.tile_pool(name="idx", bufs=2) as idxp,
            tc.tile_pool(name="ald", bufs=1) as aldp,
            tc.tile_pool(name="g", bufs=5) as gp,
            tc.tile_pool(name="m", bufs=3) as mp,
            tc.tile_pool(name="w32", bufs=6) as wp,
            tc.tile_pool(name="norm", bufs=6) as normp,
            tc.tile_pool(name="small", bufs=6) as smallp,
            tc.tile_pool(name="hkeep", bufs=1) as hkp,
            tc.tile_pool(name="cls", bufs=1) as clsp,
            tc.tile_pool(name="psd", bufs=2, space="PSUM") as psd,
            tc.tile_pool(name="pse", bufs=3, space="PSUM") as pse,
            tc.tile_pool(name="pst", bufs=2, space="PSUM") as pstp,
            tc.tile_pool(name="psc", bufs=1, space="PSUM") as pscp,
        ):
            ACT = mybir.ActivationFunctionType
            ident = constp.tile([P, P], f16)
            nc.sync.dma_start(out=ident[:, :], in_=ident_in[:, :])
            wf1 = constp.tile([F_IN, 132], f16)
            nc.sync.dma_start(out=wf1[:, :], in_=wfull1_in[:, :])
            wf2 = constp.tile([HC, 132], f16)
            nc.sync.dma_start(out=wf2[:, :], in_=wfull2_in[:, :])
            rot1 = constp.tile([HC, HC], f16, name="rot1", tag="rot1")
            nc.sync.dma_start(out=rot1[:, :], in_=rot1_in[:, :])
            rot2 = constp.tile([HC, HC], f16, name="rot2", tag="rot2")
            nc.sync.dma_start(out=rot2[:, :], in_=rot2_in[:, :])
            wc_sb = constp.tile([HC, NCLS], f16)
            nc.sync.dma_start(out=wc_sb[:, :], in_=wc_in[:, :])
            dummy_sb = constp.tile([1, 128], u16)
            nc.sync.dma_start(out=dummy_sb[:, :], in_=dummy_in[:, :])
            dummy8_sb = constp.tile([1, 128], u8)
            nc.sync.dma_start(out=dummy8_sb[:, :], in_=dummy8_in[:, :])
            b_sb = {}
            for name, hnd, w in (("b1", b1_in, HC), ("b2", b2_in, HC),
                                 ("bc", bc_in, NCLS)):
                t = constp.tile([P, w], f32, name=f"bsb_{name}", tag=f"bsb_{name}")
                nc.sync.dma_start(out=t[:, :], in_=hnd[0:1, :].to_broadcast((P, w)))
                b_sb[name] = t
            bf16_1 = constp.tile([P, HC], f16, name="b1f16", tag="b1f16")
            nc.vector.tensor_copy(out=bf16_1[:, :], in_=b_sb["b1"][:, :])
            bf16_2 = constp.tile([P, HC], f16, name="b2f16", tag="b2f16")
            nc.vector.tensor_copy(out=bf16_2[:, :], in_=b_sb["b2"][:, :])

            # dummy rows: t_full1 both ends; A row 0; B last row
            nc.sync.dma_start(out=t_full1[0:1, :], in_=dummy_sb[:, :])
            nc.sync.dma_start(
                out=t_full1[table_rows - 1:table_rows, :], in_=dummy_sb[:, :]
            )
            nc.sync.dma_start(out=t2ag[0:1, :], in_=dummy_sb[:, :])
            nc.sync.dma_start(
                out=t2ag[table_rows - 1:table_rows, :], in_=dummy_sb[:, :]
            )

            # ---- replicated dense phase for layer 1 ----
            ald1 = aldp.tile([P, tpc, 4], f32, name="ald1", tag="ald1")
            ald2 = aldp.tile([P, tpc, 4], f32, name="ald2", tag="ald2")
            BT = 3   # tiles per PSUM batch
            GB = 12  # tiles per DMA group
            for gg in range(0, n_tiles, GB):
                gcnt = min(GB, n_tiles - gg)
                xt = xtp.tile([P, gcnt * P], f16, tag="xt")
                nc.sync.dma_start(
                    out=xt[:, :], in_=xT_in[:, gg * P:(gg + gcnt) * P]
                )
                ts_ = tsp.tile([P, gcnt, 128], u16, tag="ts")
                for g0 in range(gg, gg + gcnt, BT):
                    gn = min(BT, gg + gcnt - g0)
                    o = g0 - gg
                    ps = psd.tile([P, gn, 132], f32, tag="psd")
                    for j in range(gn):
                        nc.tensor.matmul(
                            out=ps[:, j, :],
                            lhsT=xt[:, (o + j) * P:(o + j + 1) * P],
                            rhs=wf1[:, :],
                            start=True, stop=True,
                        )
                    nc.scalar.activation(
                        out=ts_[:, o:o + gn, 0:128].bitcast(f16),
                        in_=ps[:, :, 0:128],
                        func=ACT.Copy,
                    )
                    if g0 < tpc:  # own tiles: extract a_dst . h
                        jn = min(gn, tpc - g0)
                        nc.vector.tensor_copy(
                            out=ald1[:, g0:g0 + jn, :], in_=ps[:, 0:jn, 128:132]
                        )
                nc.sync.dma_start(
                    out=t_full1[1:1 + npad, :].rearrange(
                        "(p g) w -> p g w", g=n_tiles)[:, gg:gg + gcnt, :],
                    in_=ts_[:, :, :],
                )

            # ald for layer 2 is produced during layer-1 normalization.
            accA = hkp.tile([P, tpc, 132], f16, name="accA", tag="accA")

            zsAll = clsp.tile([P, tpc, NCLS], f32, name="zsAll", tag="zsAll")
            smAll = clsp.tile([P, tpc], f32, name="smAll", tag="smAll")
            lnAll = clsp.tile([P, tpc], f32, name="lnAll", tag="lnAll")
            t2b_hold = {}

            for layer in (1, 2):
                KL, KH = KLs[layer], KHs[layer]
                cumKL = np.concatenate([[0], np.cumsum(KL)])
                cumKH = np.concatenate([[0], np.cumsum(KH)])
                idx_sb = idxp.tile([128, idxcols], i16, tag="idx")
                nc.sync.dma_start(
                    out=idx_sb[:, :],
                    in_=(idx1_in if layer == 1 else idx2_in)[:, :],
                )
                t_full = t_full1
                ald = ald1 if layer == 1 else ald2
                bias = bf16_1 if layer == 1 else bf16_2
                bzero = b1_zero if layer == 1 else b2_zero

                if layer == 1:
                    KLm, KHm = KL, KH
                    cumKLm = cumKL
                    lo_base = 0
                    lo_win = t_full[0:min(WIN, table_rows), :]
                    tdt, gdt = u16, f16
                else:
                    KLm, KHm = KH, np.zeros_like(KH)  # B stream only
                    cumKLm = cumKH
                    lo_base = locol[layer]
                    lo_win = t2ag[hi_base:table_rows, :]
                    tdt, gdt = u16, f16
                    nc.vector.memset(accA[:, :, :], 0.0)
                # sub-phase A (layer 2): sources below `cut`, gathered from
                # t2ag[0:cut) as soon as all but the last AG chunk land
                def emit_a_early(r):
                    ka = int(KL[r])
                    gA = gp.tile([P, ka, 128], u16, tag="g")
                    off = 0
                    col = 8 * int(cumKL[r])
                    while off < ka:
                        jn = min(MAXCH, ka - off)
                        nc.gpsimd.dma_gather(
                            out_ap=gA[:, off:off + jn, :],
                            in_ap=t2ag[0:cutA, :],
                            idxs_ap=idx_sb[:, col:col + jn * 8],
                            num_idxs=jn * P,
                            num_idxs_reg=jn * P,
                            elem_size=128,
                        )
                        off += jn
                        col += jn * 8
                    wA = wp.tile([P, ka, 4], f32, tag="w32")
                    nc.vector.tensor_tensor(
                        out=wA[:, :, :],
                        in0=gA[:, :, :].bitcast(f16).rearrange(
                            "p k (h c) -> p k h c", h=H)[:, :, :, 0],
                        in1=ald[:, r:r + 1, :].to_broadcast((P, ka, 4)),
                        op=mybir.AluOpType.add,
                    )
                    zA = wp.tile([P, ka, 4], f32, tag="zt")
                    nc.scalar.activation(
                        out=zA[:, :, :], in_=wA[:, :, :],
                        func=ACT.Prelu, alpha=NEG,
                    )
                    mA = mp.tile([P, ka, 132], f16, tag="m")
                    nc.scalar.activation(
                        out=mA[:, :, 128:132], in_=zA[:, :, :], func=ACT.Exp,
                    )
                    aA = wp.tile([P, ka, 4, 2], f16, tag="a2")
                    nc.vector.tensor_copy(
                        out=aA[:, :, :, :],
                        in_=mA[:, :, 128:132][:, :, :, None].to_broadcast(
                            (P, ka, 4, 2)),
                    )
                    nc.vector.tensor_tensor(
                        out=mA[:, :, 0:128].rearrange(
                            "p k (h c d) -> p k h c d", h=H, d=2),
                        in0=gA[:, :, :].bitcast(f16).rearrange(
                            "p k (h c d) -> p k h c d", h=H, d=2),
                        in1=aA[:, :, :, None, :].to_broadcast(
                            (P, ka, 4, 16, 2)),
                        op=mybir.AluOpType.mult,
                    )
                    psA = pse.tile([P, 2, 132], f32, tag="pse")
                    npairs = ka // 2
                    for j in range(npairs):
                        nc.tensor.matmul(
                            out=psA[:, :, :], lhsT=ident[:, :],
                            rhs=mA[:, 2 * j:2 * j + 2, :],
                            start=(j == 0),
                            stop=(j == npairs - 1 and ka % 2 == 0),
                        )
                    if ka % 2:
                        nc.tensor.matmul(
                            out=psA[:, 0, :], lhsT=ident[:, :],
                            rhs=mA[:, ka - 1, :],
                            start=(npairs == 0), stop=True,
                        )
                    return psA

                def emit_a_late(r, psA):
                    accA0 = normp.tile([P, 132], f16, tag="acc0")
                    nc.scalar.activation(
                        out=accA0[:, :], in_=psA[:, 0, :], func=ACT.Copy,
                    )
                    nc.vector.tensor_tensor(
                        out=accA[:, r, :], in0=accA0[:, :], in1=psA[:, 1, :],
                        op=mybir.AluOpType.add,
                    )

                if layer == 2:
                    apend = []
                    for r in range(tpc):
                        if int(KL[r]) == 0:
                            continue
                        psA_r = emit_a_early(r)
                        apend.append((r, psA_r))
                        if len(apend) > 1:
                            emit_a_late(*apend.pop(0))
                    while apend:
                        emit_a_late(*apend.pop(0))
                # main per-round loop (layer 1: lo+hi; layer 2: B stream),
                # software-pipelined: round r+1's gather/mult half is emitted
                # before round r's normalize half so in-order engine queues
                # don't serialize the cross-engine tail chain between rounds.
                def emit_early(r):
                    kl, kh = int(KLm[r]), int(KHm[r])
                    K = kl + kh
                    g = gp.tile([P, K, 128], tdt, tag="g")
                    # lo gather
                    off = 0
                    col = lo_base + 8 * int(cumKLm[r])
                    while off < kl:
                        jn = min(MAXCH, kl - off)
                        nc.gpsimd.dma_gather(
                            out_ap=g[:, off:off + jn, :],
                            in_ap=lo_win,
                            idxs_ap=idx_sb[:, col:col + jn * 8],
                            num_idxs=jn * P,
                            num_idxs_reg=jn * P,
                            elem_size=128,
                        )
                        off += jn
                        col += jn * 8
                    # hi gather
                    off = 0
                    col = locol[layer] + 8 * int(cumKH[r])
                    while off < kh:
                        jn = min(MAXCH, kh - off)
                        nc.gpsimd.dma_gather(
                            out_ap=g[:, kl + off:kl + off + jn, :],
                            in_ap=t_full[hi_base:table_rows, :],
                            idxs_ap=idx_sb[:, col:col + jn * 8],
                            num_idxs=jn * P,
                            num_idxs_reg=jn * P,
                            elem_size=128,
                        )
                        off += jn
                        col += jn * 8

                    # w = exp(prelu(als_src + ald_dst))
                    w32 = wp.tile([P, K, 4], f32, tag="w32")
                    nc.vector.tensor_tensor(
                        out=w32[:, :, :],
                        in0=g[:, :, :].bitcast(gdt).rearrange(
                            "p k (h c) -> p k h c", h=H)[:, :, :, 0],
                        in1=ald[:, r:r + 1, :].to_broadcast((P, K, 4)),
                        op=mybir.AluOpType.add,
                    )
                    zt = wp.tile([P, K, 4], f32, tag="zt")
                    if NO_PRELU:
                        nc.vector.tensor_scalar_mul(
                            out=zt[:, :, :], in0=w32[:, :, :], scalar1=NEG,
                        )
                        nc.vector.tensor_tensor(
                            out=zt[:, :, :], in0=w32[:, :, :], in1=zt[:, :, :],
                            op=mybir.AluOpType.max,
                        )
                    else:
                        nc.scalar.activation(
                            out=zt[:, :, :], in_=w32[:, :, :],
                            func=ACT.Prelu, alpha=NEG,
                        )
                    m = mp.tile([P, K, 132], f16, tag="m")
                    nc.scalar.activation(
                        out=m[:, :, 128:132], in_=zt[:, :, :], func=ACT.Exp,
                    )
                    a2 = wp.tile([P, K, 4, 2], f16, tag="a2")
                    nc.vector.tensor_copy(
                        out=a2[:, :, :, :],
                        in_=m[:, :, 128:132][:, :, :, None].to_broadcast(
                            (P, K, 4, 2)),
                    )
                    nc.vector.tensor_tensor(
                        out=m[:, :, 0:128].rearrange(
                            "p k (h c d) -> p k h c d", h=H, d=2),
                        in0=g[:, :, :].bitcast(gdt).rearrange(
                            "p k (h c d) -> p k h c d", h=H, d=2),
                        in1=a2[:, :, :, None, :].to_broadcast((P, K, 4, 16, 2)),
                        op=mybir.AluOpType.mult,
                    )
                    # accumulate the round in PSUM via identity matmuls,
                    # two chunks per matmul (halves PE instruction count)
                    ps = pse.tile([P, 2, 132], f32, tag="pse")
                    if NO_PAIR:
                        nc.vector.memset(ps[:, 1, :], 0.0)
                        for j in range(K):
                            nc.tensor.matmul(
                                out=ps[:, 0, :], lhsT=ident[:, :], rhs=m[:, j, :],
                                start=(j == 0), stop=(j == K - 1),
                            )
                    else:
                        npairs = K // 2
                        for j in range(npairs):
                            nc.tensor.matmul(
                                out=ps[:, :, :], lhsT=ident[:, :],
                                rhs=m[:, 2 * j:2 * j + 2, :],
                                start=(j == 0),
                                stop=(j == npairs - 1 and K % 2 == 0),
                            )
                        if K % 2:
                            nc.tensor.matmul(
                                out=ps[:, 0, :], lhsT=ident[:, :],
                                rhs=m[:, K - 1, :],
                                start=False, stop=True,
                            )
                    acc0 = normp.tile([P, 132], f16, tag="acc0")
                    nc.scalar.activation(
                        out=acc0[:, :], in_=ps[:, 0, :], func=ACT.Copy,
                    )
                    accf = normp.tile([P, 132], f16, tag="accf")
                    nc.vector.tensor_tensor(
                        out=accf[:, :], in0=acc0[:, :], in1=ps[:, 1, :],
                        op=mybir.AluOpType.add,
                    )
                    if layer == 2:
                        accfB = normp.tile([P, 132], f16, tag="accfB")
                        nc.vector.tensor_tensor(
                            out=accfB[:, :], in0=accf[:, :],
                            in1=accA[:, r, :],
                            op=mybir.AluOpType.add,
                        )
                        accf = accfB
                    return accf

                def emit_late(r, accf):
                    # normalize + bias + leaky
                    den = smallp.tile([P, 4], f16, tag="den")
                    nc.vector.tensor_scalar_add(
                        out=den[:, :], in0=accf[:, 128:132], scalar1=1e-4,
                    )
                    rc2 = smallp.tile([P, 4, 2], f16, tag="rc2")
                    with nc.allow_low_precision(reason="f16 softmax denom"):
                        nc.vector.reciprocal(out=rc2[:, :, 0], in_=den[:, :])
                    nc.vector.tensor_copy(
                        out=rc2[:, :, 1], in_=rc2[:, :, 0],
                    )
                    xn = normp.tile([P, HC], f16, tag="xn")
                    nc.vector.tensor_tensor(
                        out=xn[:, :].rearrange("p (h c d) -> p h c d", h=H, d=2),
                        in0=accf[:, 0:128].rearrange(
                            "p (h c d) -> p h c d", h=H, d=2),
                        in1=rc2[:, :, None, :].to_broadcast((P, 4, 16, 2)),
                        op=mybir.AluOpType.mult,
                    )
                    ptry = pstp.tile([P, P], f16, tag="pst")
                    nc.tensor.transpose(
                        out=ptry[:, :], in_=xn[:, :], identity=ident[:, :]
                    )
                    yT = normp.tile([P, P], f16, tag="yT")
                    nc.scalar.activation(
                        out=yT[:, :], in_=ptry[:, :], func=ACT.Copy,
                    )
                    prot = pstp.tile([P, HC], f32, tag="pst")
                    nc.tensor.matmul(
                        out=prot[:, :], lhsT=yT[:, :],
                        rhs=(rot1 if layer == 1 else rot2)[:, :],
                        start=True, stop=True,
                    )
                    xh = normp.tile([P, HC], f16, tag="xh")
                    nc.scalar.activation(
                        out=xh[:, :], in_=prot[:, :], func=ACT.Copy,
                    )
                    if not bzero:
                        nc.vector.tensor_tensor(
                            out=xh[:, :], in0=xh[:, :], in1=bias[:, :],
                            op=mybir.AluOpType.add,
                        )
                    hnext = normp.tile([P, HC], f16, tag="hnext")
                    if NO_PRELU:
                        nc.vector.tensor_scalar_mul(
                            out=hnext[:, :], in0=xh[:, :], scalar1=NEG,
                        )
                        nc.vector.tensor_tensor(
                            out=hnext[:, :], in0=xh[:, :], in1=hnext[:, :],
                            op=mybir.AluOpType.max,
                        )
                    else:
                        nc.scalar.activation(
                            out=hnext[:, :], in_=xh[:, :], func=ACT.Prelu,
                            alpha=NEG,
                        )

                    ptr = pstp.tile([P, P], f16, tag="pst")
                    nc.tensor.transpose(
                        out=ptr[:, :], in_=hnext[:, :], identity=ident[:, :]
                    )
                    hT = normp.tile([P, P], f16, tag="hT")
                    nc.scalar.activation(
                        out=hT[:, :], in_=ptr[:, :], func=ACT.Copy,
                    )
                    if layer == 1:
                        ps2 = psd.tile([P, 1, 132], f32, tag="psd")
                        nc.tensor.matmul(
                            out=ps2[:, 0, :], lhsT=hT[:, :], rhs=wf2[:, :],
                            start=True, stop=True,
                        )
                        kch = int(np.searchsorted(agb, r, side="right")) - 1
                        rr = r - int(agb[kch])
                        tk = int(agb[kch + 1]) - int(agb[kch])
                        # tiles per t2loc store (contig per partition); the
                        # first chunk flushes eagerly so AG0 fires sooner
                        SB = 2 if kch == 0 else 4
                        if rr % SB == 0:
                            t2b = tsp.tile(
                                [P, min(SB, tk - rr), 128], u16,
                                name="t2b", tag="t2s")
                            t2b_hold[0] = t2b
                        t2b = t2b_hold[0]
                        nc.scalar.activation(
                            out=t2b[:, rr % SB, :].bitcast(f16),
                            in_=ps2[:, 0, 0:128], func=ACT.Copy,
                        )
                        nc.vector.tensor_copy(
                            out=ald2[:, r, :], in_=ps2[:, 0, 128:132]
                        )
                        if rr % SB == SB - 1 or rr == tk - 1:
                            cnt = rr % SB + 1
                            r0 = rr - cnt + 1
                            nc.sync.dma_start(
                                out=t2loc[kch].rearrange(
                                    "(p t) w -> p t w", t=tk)[:, r0:r0 + cnt, :],
                                in_=t2b[:, 0:cnt, :],
                            )
                        if r == int(agb[kch + 1]) - 1:
                            nrows = tk * P
                            base = 1 + sum(
                                (int(agb[j + 1]) - int(agb[j])) * P * NCORES
                                for j in range(kch))
                            gtot = nrows * NCORES
                            agout = t2ag[base:base + gtot, :]
                            nc.gpsimd.collective_compute(
                                "AllGather",
                                mybir.AluOpType.bypass,
                                ins=[t2loc[kch][:, :]],
                                outs=[agout],
                                replica_groups=rg,
                            )
                    else:
                        pc = pscp.tile([P, NCLS], f32, tag="pc")
                        nc.tensor.matmul(
                            out=pc[:, :], lhsT=hT[:, :], rhs=wc_sb[:, :],
                            start=True, stop=True,
                        )
                        lg = normp.tile([P, NCLS], f32, tag="lg")
                        nc.scalar.activation(
                            out=lg[:, :], in_=pc[:, :], func=ACT.Copy,
                        )
                        nc.vector.tensor_tensor(
                            out=lg[:, :], in0=lg[:, :], in1=b_sb["bc"][:, :],
                            op=mybir.AluOpType.add,
                        )
                        mx = smallp.tile([P, 1], f32, tag="mx")
                        nc.vector.reduce_max(
                            out=mx[:, :], in_=lg[:, :], axis=mybir.AxisListType.X
                        )
                        nc.vector.tensor_scalar(
                            out=zsAll[:, r, :], in0=lg[:, :], scalar1=mx[:, :],
                            scalar2=None, op0=mybir.AluOpType.subtract,
                        )
                        es = normp.tile([P, NCLS], f32, tag="lg2")
                        nc.scalar.activation(
                            out=es[:, :], in_=zsAll[:, r, :], func=ACT.Exp,
                        )
                        nc.vector.reduce_sum(
                            out=smAll[:, r:r + 1], in_=es[:, :],
                            axis=mybir.AxisListType.X,
                        )

                DEPTH = 1
                pend = []
                for r in range(tpc):
                    acc_r = emit_early(r)
                    pend.append((r, acc_r))
                    if len(pend) > DEPTH:
                        emit_late(*pend.pop(0))
                while pend:
                    emit_late(*pend.pop(0))

            # ---- batched log-softmax tail ----
            nc.scalar.activation(
                out=lnAll[:, :], in_=smAll[:, :], func=ACT.Ln,
            )
            otAll = clsp.tile([P, tpc, NCLS], f32, name="otAll", tag="otAll")
            for r in range(tpc):
                nc.vector.tensor_scalar(
                    out=otAll[:, r, :], in0=zsAll[:, r, :],
                    scalar1=lnAll[:, r:r + 1], scalar2=None,
                    op0=mybir.AluOpType.subtract,
                )
            nc.sync.dma_start(
                out=logits_out[:, :].rearrange("(l r) c -> l r c", r=tpc),
                in_=otAll[:, :, :],
            )

    nc.finalize()
    return nc


# ---------------------------------------------------------------------------
# entry point
# ---------------------------------------------------------------------------

_CACHE = {}


def kernel(x, edge_index, W1, a1_src, a1_dst, b1, W2, a2_src, a2_dst, b2, Wc, bc):
    global LAST_EXEC_NS
    import os

    x = np.asarray(x, dtype=np.float32)
    n_real = x.shape[0]
    b1 = np.asarray(b1, dtype=np.float32)
    b2 = np.asarray(b2, dtype=np.float32)
    b1_zero = bool(np.all(b1 == 0))
    b2_zero = bool(np.all(b2 == 0))
    ekey = hash(np.asarray(edge_index).tobytes()) ^ hash((n_real, b1_zero, b2_zero))
    if ekey in _CACHE:
        nc, st = _CACHE[ekey]
    else:
        st = _preprocess(x, edge_index, n_real)
        nc = _build(st, b1_zero, b2_zero)
        _CACHE[ekey] = (nc, st)

    npad, npc, tpc = st["npad"], st["npc"], st["tpc"]
    new_id = st["new_id"]

    # per-core rotated xT (full table, f16)
    x_pad = np.zeros((npad, F_IN), dtype=np.float32)
    x_pad[new_id[:n_real]] = x
    x_blocks = x_pad.reshape(NCORES, npc, F_IN)

    wfull1, rot1 = _wfull(W1, a1_src, a1_dst)
    wfull2, rot2 = _wfull(W2, a2_src, a2_dst)
    wc = np.ascontiguousarray(np.asarray(Wc, dtype=np.float16))
    b1r = b1[None, :]
    b2r = b2[None, :]
    bcr = np.asarray(bc, dtype=np.float32)[None, :]
    dummy = _dummy_row()
    dummy8 = _dummy_row8()
    ident = np.eye(P, dtype=np.float16)

    idxcols = max(st["idx1"].shape[2], st["idx2"].shape[2], 16)

    def pad_idx(a):
        if a.shape[2] < idxcols:
            a = np.concatenate(
                [a, np.zeros((NCORES, 128, idxcols - a.shape[2]), np.int16)],
                axis=2)
        return a

    idx1 = pad_idx(st["idx1"])
    idx2 = pad_idx(st["idx2"])

    in_maps = []
    for c in range(NCORES):
        rot = np.roll(np.arange(NCORES), -c)  # own block first
        xT = np.ascontiguousarray(
            x_blocks[rot].reshape(npad, F_IN).T.astype(np.float16)
        )
        in_maps.append({
            "xT": xT,
            "idx1": np.ascontiguousarray(idx1[c]),
            "idx2": np.ascontiguousarray(idx2[c]),
            "wfull1": wfull1, "wfull2": wfull2, "wc": wc,
            "rot1": rot1, "rot2": rot2,
            "b1": b1r, "b2": b2r, "bc": bcr,
            "dummyrow": dummy, "dummyrow8": dummy8, "ident16": ident,
        })

    os.environ.setdefault("BASS_NEVER_TRACE", "1")
    res = run_bass_kernel_spmd(nc, in_maps, core_ids=list(range(NCORES)))
    LAST_EXEC_NS = res.exec_time_ns

    # device rows are p-major (row = lane*tpc + r); restore node order
    logits_pad = np.concatenate(
        [res.results[c]["logits"].reshape(P, tpc, NCLS)
         .transpose(1, 0, 2).reshape(npc, NCLS)
         for c in range(NCORES)], axis=0
    )
    return logits_pad[new_id[:n_real]].astype(np.float32)

